# revision 1
# baseline (speedup 1.0000x reference)
"""Trainium2 Bass kernel for nn_Branch1_block (gnn_message_passing).

Data-parallel over batch on 8 NeuronCores (4 batches/core). Chebyshev
graph-conv done as tiled PE matmuls in float32r (full-rate fp32 variant);
cheb[0] == I is exploited (its graph matmul is skipped; verified on host).

Pipeline per core:
  SE attention (tiny matmul chain) -> scale folded into gconv1 z-copies
  gconv1: z_k = x @ cheb_k (k=1,2), feature mix via z.T @ blockdiag(theta1)
          producing xg1T [n x (bt,o)] directly; PE-transposes give xg1.
  gconv2: z2_k = xg1 @ cheb_k (lhsT = xg1T), feature mix blockdiag(theta2)
  tail:   temporal convs as block-structured matmuls + 1x1 residual conv,
          LayerNorm over nodes, relu.
"""
import sys

import numpy as np

try:
    import concourse.bass as bass
except ImportError:  # pragma: no cover - grading env fallback
    for p in ("/root/.axon_site", "/root/.axon_site/_ro/trn_rl_repo",
              "/root/.axon_site/_ro/pypackages", "/opt/trn_rl_repo"):
        if p not in sys.path:
            sys.path.append(p)
    import concourse.bass as bass

from contextlib import ExitStack

import concourse.mybir as mybir
import concourse.tile as tile
from concourse import bacc
from concourse.bass_utils import run_bass_kernel_spmd

B, T, F, O, N, K = 32, 12, 16, 32, 2048, 3
NCORES = 8
BC = B // NCORES          # 4 batches per core
BT = BC * T               # 48
R1 = BT * F               # 768 rows (bt,f)
R2 = BT * O               # 1536 rows (bt,o)
J1 = R1 // 128            # 6
J2 = R2 // 128            # 12
NT = N // 128             # 16
NCH = 512
NNC = N // NCH            # 4
NTL = NCH // 128          # 4 n-tiles per chunk

f32 = mybir.dt.float32
f32r = mybir.dt.float32r
AF = mybir.ActivationFunctionType
AX = mybir.AxisListType
ALU = mybir.AluOpType

# (jt, ji) pairs with nonzero temporal-conv block matrices
TC_PAIRS = {0: (0, 2), 1: (0, 1), 2: (1, 2)}
# per j1 (r1 tile): the one or two batches its rows touch
B01 = []
for _j in range(6):
    _bs = sorted({(8 * _j + _p // 16) // 12 for _p in range(128)})
    B01.append((_bs[0], _bs[-1]))

_compiled = None
PHASES = 99  # analysis knob: 1=gconv1, 2=+gconv2, 3=+tail
SKIP_FEATURE1 = False   # analysis: skip gconv1 feature mix/transposes/spills
SKIP_SPILLS = False     # analysis: skip xg1T/xg1 spill DMA writes
ZPS_BUFS = 3


def _build():
    nc = bacc.Bacc("TRN2", target_bir_lowering=False, debug=False)

    xT_d = nc.dram_tensor("xT", [N, R1], f32, kind="ExternalInput").ap()
    xb_d = nc.dram_tensor("xb", [R1, N], f32, kind="ExternalInput").ap()
    cheb_d = nc.dram_tensor("cheb1", [N, N], f32, kind="ExternalInput").ap()
    th1_d = nc.dram_tensor("th1", [3, 128, 256], f32, kind="ExternalInput").ap()
    th2_d = nc.dram_tensor("th2", [3, 128, 128], f32, kind="ExternalInput").ap()
    tcbd_d = nc.dram_tensor("tcbd", [2, 3, 3, 128, 128], f32, kind="ExternalInput").ap()
    resbd_d = nc.dram_tensor("resbd", [2, 128, 128], f32, kind="ExternalInput").ap()
    ident_d = nc.dram_tensor("ident", [128, 128], f32, kind="ExternalInput").ap()
    selA_d = nc.dram_tensor("selA", [6, 128, 48], f32, kind="ExternalInput").ap()
    selE1_d = nc.dram_tensor("selE1", [6, 2, 12, 128], f32, kind="ExternalInput").ap()
    w1aug_d = nc.dram_tensor("w1aug", [13, 3], f32, kind="ExternalInput").ap()
    w2aug_d = nc.dram_tensor("w2aug", [4, 12], f32, kind="ExternalInput").ap()
    lng_d = nc.dram_tensor("lng", [128, N], f32, kind="ExternalInput").ap()
    lnb_d = nc.dram_tensor("lnb", [128, N], f32, kind="ExternalInput").ap()
    bias3_d = nc.dram_tensor("bias3", [128, 4], f32, kind="ExternalInput").ap()
    y_d = nc.dram_tensor("y", [R2, N], f32, kind="ExternalOutput").ap()

    xg2_s = [nc.dram_tensor(f"xg2_s{b}", [3 * 128, N], f32).ap() for b in range(BC)]

    dma = nc.sync.dma_start

    with tile.TileContext(nc) as tc, ExitStack() as top:
        cpool = top.enter_context(tc.tile_pool(name="const", bufs=1))
        th2_sb = cpool.tile([128, 3 * 128], f32r)
        attc1 = cpool.tile([128, J1], f32)
        ident_sb = cpool.tile([128, 128], f32)
        ident_sbr2 = cpool.tile([128, 128], f32r)
        dma(th2_sb[:].rearrange("p (k c) -> p k c", c=128),
            th2_d.rearrange("k p c -> p k c").bitcast(f32r))
        dma(ident_sb[:], ident_d)
        dma(ident_sbr2[:], ident_d.bitcast(f32r))

        xg1Tp = tc.tile_pool(name="xg1T", bufs=1)
        xg1Tpool = xg1Tp.__enter__()
        xg1T_sb = xg1Tpool.tile([128, NT, R2], f32r)

        c1 = tc.tile_pool(name="c1", bufs=1)
        c1pool = c1.__enter__()
        th1_sb = c1pool.tile([128, 3 * 256], f32r)
        selA_sb = c1pool.tile([128, 6 * 48], f32)
        selE1_sb = c1pool.tile([12, 12 * 128], f32)
        w1aug_sb = c1pool.tile([13, 3], f32)
        w2aug_sb = c1pool.tile([4, 12], f32)
        dma(th1_sb[:].rearrange("p (k c) -> p k c", c=256),
            th1_d.rearrange("k p c -> p k c").bitcast(f32r))
        dma(selA_sb[:].rearrange("p (j s) -> p j s", s=48),
            selA_d.rearrange("j p s -> p j s"))
        dma(selE1_sb[:].rearrange("p (i q) -> p i q", q=128),
            selE1_d.rearrange("a b p q -> p (a b) q"))
        dma(w1aug_sb[:], w1aug_d)
        dma(w2aug_sb[:], w2aug_d)

        # ---------- gconv1 ----------
        NCH1 = 256
        NNC1 = N // NCH1
        with tc.tile_pool(name="xTp", bufs=1) as xTpool, \
             tc.tile_pool(name="chp", bufs=4) as chpool, \
             tc.tile_pool(name="g1sb", bufs=2) as g1pool, \
             tc.tile_pool(name="attps", bufs=1, space="PSUM") as apsum, \
             tc.tile_pool(name="attsb", bufs=2) as aspool, \
             tc.tile_pool(name="zps", bufs=3, space="PSUM") as zpsum, \
             tc.tile_pool(name="fps", bufs=2, space="PSUM") as fpsum:
            xT_sb = xTpool.tile([128, NT, R1], f32r)
            xTv = xT_d.rearrange("(mi p) r -> mi p r", p=128).bitcast(f32r)
            for q in range(4):
                dma(xT_sb[:, q * 4:(q + 1) * 4, :],
                    xTv[q * 4:(q + 1) * 4].rearrange("mi p r -> p mi r"))

            # ---- SE attention (from xT row-sums) ----
            ones_r = aspool.tile([128, 1], f32, tag="ones")
            nc.vector.memset(ones_r[:], 1.0)
            rs = aspool.tile([128, J1], f32, tag="rs")
            rsps = apsum.tile([128, J1], f32, tag="attp")
            for j in range(J1):
                for mi in range(NT):
                    nc.tensor.matmul(rsps[:, j:j + 1],
                                     xT_sb[:, mi, j * 128:(j + 1) * 128].bitcast(f32),
                                     ones_r[:], start=(mi == 0), stop=(mi == NT - 1))
            nc.vector.tensor_copy(rs[:], rsps[:])
            att0ps = apsum.tile([48, 1], f32, tag="attp")
            for j in range(J1):
                nc.tensor.matmul(att0ps[:], selA_sb[:, j * 48:(j + 1) * 48],
                                 rs[:, j:j + 1], start=(j == 0), stop=(j == J1 - 1))
            att0sb = aspool.tile([48, 1], f32, tag="att0")
            nc.scalar.activation(att0sb[:], att0ps[:], AF.Copy, scale=1.0 / (F * N))
            atbps = apsum.tile([12, 4], f32, tag="attp")
            for b in range(4):
                nc.tensor.matmul(atbps[:, b:b + 1],
                                 ident_sb[:48, b * 12:(b + 1) * 12],
                                 att0sb[:], start=True, stop=True)
            atb13 = aspool.tile([13, 4], f32, tag="atb13")
            nc.vector.memset(atb13[:], 1.0)
            nc.scalar.activation(atb13[:12, :], atbps[:], AF.Copy)
            a1ps = apsum.tile([3, 4], f32, tag="attp")
            nc.tensor.matmul(a1ps[:], w1aug_sb[:], atb13[:], start=True, stop=True)
            a1sb = aspool.tile([4, 4], f32, tag="a1")
            nc.vector.memset(a1sb[:], 1.0)
            nc.scalar.activation(a1sb[:3, :], a1ps[:], AF.Relu)
            attps2 = apsum.tile([12, 4], f32, tag="attp")
            nc.tensor.matmul(attps2[:], w2aug_sb[:], a1sb[:], start=True, stop=True)
            att_tb = aspool.tile([12, 4], f32, tag="att_tb")
            nc.scalar.activation(att_tb[:], attps2[:], AF.Sigmoid)
            for j in range(J1):
                b0, b1 = B01[j]
                acps = apsum.tile([128, 1], f32, tag="attp")
                nc.tensor.matmul(acps[:], selE1_sb[:, (j * 2) * 128:(j * 2 + 1) * 128],
                                 att_tb[:, b0:b0 + 1], start=True, stop=False)
                nc.tensor.matmul(acps[:], selE1_sb[:, (j * 2 + 1) * 128:(j * 2 + 2) * 128],
                                 att_tb[:, b1:b1 + 1], start=False, stop=True)
                nc.scalar.activation(attc1[:, j:j + 1], acps[:], AF.Copy)

            # ---- graph + feature ----
            for nci in range(NNC1):
                ncs = nci * NCH1
                ch = chpool.tile([128, 4, NCH1], f32r, tag="ch", bufs=4,
                                 name=f"ch1_{nci}")
                ch2 = chpool.tile([128, 4, NCH1], f32r, tag="ch", bufs=4,
                                  name=f"ch1b_{nci}")
                chv = cheb_d[:, ncs:ncs + NCH1].rearrange("(mi p) n -> mi p n",
                                                          p=128).bitcast(f32r)
                dma(ch[:], chv[0:4].rearrange("mi p n -> p mi n"))
                dma(ch2[:], chv[4:8].rearrange("mi p n -> p mi n"))
                ch3 = chpool.tile([128, 4, NCH1], f32r, tag="ch", bufs=4,
                                  name=f"ch1c_{nci}")
                ch4 = chpool.tile([128, 4, NCH1], f32r, tag="ch", bufs=4,
                                  name=f"ch1d_{nci}")
                dma(ch3[:], chv[8:12].rearrange("mi p n -> p mi n"))
                dma(ch4[:], chv[12:16].rearrange("mi p n -> p mi n"))
                chq = [ch, ch2, ch3, ch4]

                def chslice(mi):
                    return chq[mi // 4][:, mi % 4, :]

                zk = {}
                for k in (1, 2):
                    if k == 2:
                        for mi in range(NT):
                            cmi = chslice(mi)
                            nc.vector.scalar_tensor_tensor(cmi, cmi.bitcast(f32), 2.0,
                                                           cmi.bitcast(f32),
                                                           ALU.mult, ALU.mult)
                            if mi * 128 >= ncs and mi * 128 < ncs + NCH1:
                                off = mi * 128 - ncs
                                nc.vector.tensor_sub(cmi[:, off:off + 128],
                                                     cmi[:, off:off + 128].bitcast(f32),
                                                     ident_sb[:])
                    for j1 in range(J1):
                        zps = zpsum.tile([128, NCH1], f32)
                        for mi in range(NT):
                            nc.tensor.matmul(zps[:],
                                             xT_sb[:, mi, j1 * 128:(j1 + 1) * 128],
                                             chslice(mi),
                                             start=(mi == 0), stop=(mi == NT - 1))
                        zt = g1pool.tile([128, NCH1], f32r, tag=f"z{k}",
                                         bufs=(7 if k == 1 else 3))
                        nc.vector.tensor_scalar_mul(zt[:], zps[:], attc1[:, j1:j1 + 1])
                        zk[(k, j1)] = zt
                        if k == 2:
                            z0t = g1pool.tile([128, NCH1], f32r, tag="z0", bufs=3)
                            dma(z0t[:], xb_d[j1 * 128:(j1 + 1) * 128,
                                             ncs:ncs + NCH1].bitcast(f32r))
                            nc.vector.tensor_scalar_mul(z0t[:], z0t[:].bitcast(f32),
                                                        attc1[:, j1:j1 + 1])
                            for ntl in range(NCH1 // 128):
                                ntile = (ncs // 128) + ntl
                                fps = fpsum.tile([128, 256], f32)
                                nc.tensor.matmul(fps[:], z0t[:, ntl * 128:(ntl + 1) * 128],
                                                 th1_sb[:, 0:256], start=True, stop=False)
                                nc.tensor.matmul(fps[:],
                                                 zk[(1, j1)][:, ntl * 128:(ntl + 1) * 128],
                                                 th1_sb[:, 256:512],
                                                 start=False, stop=False)
                                nc.tensor.matmul(fps[:],
                                                 zk[(2, j1)][:, ntl * 128:(ntl + 1) * 128],
                                                 th1_sb[:, 512:768],
                                                 start=False, stop=True)
                                nc.scalar.activation(
                                    xg1T_sb[:, ntile, j1 * 256:(j1 + 1) * 256],
                                    fps[:], AF.Relu)

        c1.__exit__(None, None, None)
        if PHASES < 2:
            xg1Tp.__exit__(None, None, None)
            nc.compile()
            return nc

        # ---------- gconv2 ----------
        with tc.tile_pool(name="chp2", bufs=2) as chpool2, \
             tc.tile_pool(name="g2sb", bufs=2) as g2pool, \
             tc.tile_pool(name="zps2", bufs=3, space="PSUM") as zpsum2, \
             tc.tile_pool(name="tp2", bufs=2, space="PSUM") as tpsum2, \
             tc.tile_pool(name="fps2", bufs=2, space="PSUM") as fpsum2:
            for nci in range(NNC):
                ncs = nci * NCH
                ch = chpool2.tile([128, NT, NCH], f32r, tag="ch2", bufs=2)
                chv = cheb_d[:, ncs:ncs + NCH].rearrange("(mi p) n -> mi p n",
                                                         p=128).bitcast(f32r)
                for q in range(4):
                    dma(ch[:, q * 4:(q + 1) * 4, :],
                        chv[q * 4:(q + 1) * 4].rearrange("mi p n -> p mi n"))
                z1l = {}
                for k in (1, 2):
                    if k == 2:
                        for mi in range(NT):
                            cmi = ch[:, mi, :]
                            nc.vector.scalar_tensor_tensor(cmi, cmi.bitcast(f32), 2.0,
                                                           cmi.bitcast(f32),
                                                           ALU.mult, ALU.mult)
                            if nci * NTL <= mi < nci * NTL + NTL:
                                off = mi * 128 - ncs
                                nc.vector.tensor_sub(ch[:, mi, off:off + 128],
                                                     ch[:, mi, off:off + 128].bitcast(f32),
                                                     ident_sb[:])
                    for j2 in range(J2):
                        zps = zpsum2.tile([128, NCH], f32)
                        for mi in range(NT):
                            nc.tensor.matmul(zps[:],
                                             xg1T_sb[:, mi, j2 * 128:(j2 + 1) * 128],
                                             ch[:, mi, :],
                                             start=(mi == 0), stop=(mi == NT - 1))
                        zt = g2pool.tile([128, NCH], f32r, tag=f"z2_{k}",
                                         bufs=(13 if k == 1 else 3))
                        nc.vector.tensor_copy(zt[:], zps[:])
                        if k == 1:
                            z1l[j2] = zt
                        else:
                            xg1rhs = g2pool.tile([128, NCH], f32r, tag="xg1rhs", bufs=3)
                            for ntl in range(NTL):
                                mi = nci * NTL + ntl
                                tp = tpsum2.tile([128, 128], f32r)
                                nc.tensor.transpose(
                                    tp[:], xg1T_sb[:, mi, j2 * 128:(j2 + 1) * 128],
                                    ident_sbr2[:])
                                nc.vector.tensor_copy(
                                    xg1rhs[:, ntl * 128:(ntl + 1) * 128], tp[:])
                            fps = fpsum2.tile([128, NCH], f32)
                            nc.tensor.matmul(fps[:], th2_sb[:, 0:128], xg1rhs[:],
                                             start=True, stop=False)
                            nc.tensor.matmul(fps[:], th2_sb[:, 128:256], z1l[j2][:],
                                             start=False, stop=False)
                            nc.tensor.matmul(fps[:], th2_sb[:, 256:384], zt[:],
                                             start=False, stop=True)
                            xg2t = g2pool.tile([128, NCH], f32r, tag="xg2t", bufs=3)
                            nc.scalar.activation(xg2t[:], fps[:], AF.Relu)
                            dma(xg2_s[j2 // 3][(j2 % 3) * 128:(j2 % 3 + 1) * 128,
                                      ncs:ncs + NCH].bitcast(f32r), xg2t[:])

        xg1Tp.__exit__(None, None, None)

        if PHASES < 3:
            nc.compile()
            return nc
        # ---------- tail: tconv x2 + residual + LayerNorm + relu ----------
        with tc.tile_pool(name="c3", bufs=1) as c3pool, \
             tc.tile_pool(name="tlsb", bufs=8) as tlpool, \
             tc.tile_pool(name="tlbig", bufs=2) as tbpool, \
             tc.tile_pool(name="tstat", bufs=8) as stpool, \
             tc.tile_pool(name="tps1", bufs=2, space="PSUM") as tpsum1, \
             tc.tile_pool(name="tps2", bufs=2, space="PSUM") as tpsum2, \
             tc.tile_pool(name="rps", bufs=2, space="PSUM") as rpsum:
            tcbd_sb = c3pool.tile([128, 18 * 128], f32r)
            resbd_sb = c3pool.tile([128, 2 * 128], f32r)
            lng_sb = c3pool.tile([128, N], f32)
            lnb_sb = c3pool.tile([128, N], f32)
            bias3_sb = c3pool.tile([128, 4], f32)
            first_loads = []
            for i in range(3):
                t_ = tbpool.tile([128, N], f32r, tag="xg2b", bufs=5,
                                 name=f"xg2b_0_{i}")
                dma(t_[:], xg2_s[0][i * 128:(i + 1) * 128, :].bitcast(f32r))
                first_loads.append(t_)
            dma(tcbd_sb[:].rearrange("p (i q) -> p i q", q=128),
                tcbd_d.rearrange("a b c p q -> p (a b c) q").bitcast(f32r))
            dma(resbd_sb[:].rearrange("p (h q) -> p h q", q=128),
                resbd_d.rearrange("h p q -> p h q").bitcast(f32r))
            dma(bias3_sb[:], bias3_d)
            dma(lng_sb[:], lng_d)
            dma(lnb_sb[:], lnb_d)
            for b in range(BC):
                if b == 0:
                    xg2b = first_loads
                else:
                    xg2b = []
                    for i in range(3):
                        t_ = tbpool.tile([128, N], f32r, tag="xg2b", bufs=5,
                                         name=f"xg2b_{b}_{i}")
                        dma(t_[:], xg2_s[b][i * 128:(i + 1) * 128, :].bitcast(f32r))
                        xg2b.append(t_)
                xt1b = [tbpool.tile([128, N], f32r, tag="xt1b", name=f"xt1b_{b}_{i}",
                                    bufs=5)
                        for i in range(3)]
                for jt in range(3):
                    ja, jb = TC_PAIRS[jt]
                    for nci in range(NNC):
                        ncs = nci * NCH
                        tp1 = tpsum1.tile([128, NCH], f32)
                        ca = ((0 * 3 + jt) * 3 + ja) * 128
                        cb = ((0 * 3 + jt) * 3 + jb) * 128
                        nc.tensor.matmul(tp1[:], tcbd_sb[:, ca:ca + 128],
                                         xg2b[ja][:, ncs:ncs + NCH],
                                         start=True, stop=False)
                        nc.tensor.matmul(tp1[:], tcbd_sb[:, cb:cb + 128],
                                         xg2b[jb][:, ncs:ncs + NCH],
                                         start=False, stop=True)
                        nc.scalar.activation(xt1b[jt][:, ncs:ncs + NCH], tp1[:],
                                             AF.Relu, bias=bias3_sb[:, 0:1])
                for jt in range(3):
                    j2 = 3 * b + jt
                    ja, jb = TC_PAIRS[jt]
                    yfull = tbpool.tile([128, N], f32, tag="yfull", bufs=3,
                                        name=f"yf_{b}_{jt}")
                    for nci in range(NNC):
                        ncs = nci * NCH
                        tp2 = tpsum2.tile([128, NCH], f32)
                        ca = ((1 * 3 + jt) * 3 + ja) * 128
                        cb = ((1 * 3 + jt) * 3 + jb) * 128
                        nc.tensor.matmul(tp2[:], tcbd_sb[:, ca:ca + 128],
                                         xt1b[ja][:, ncs:ncs + NCH],
                                         start=True, stop=False)
                        nc.tensor.matmul(tp2[:], tcbd_sb[:, cb:cb + 128],
                                         xt1b[jb][:, ncs:ncs + NCH],
                                         start=False, stop=True)
                        xt2c = tlpool.tile([128, NCH], f32, tag="xt2c")
                        nc.scalar.activation(xt2c[:], tp2[:], AF.Relu,
                                             bias=bias3_sb[:, 1:2])
                        jx, half = divmod(j2, 2)
                        xres = tlpool.tile([128, NCH], f32r, tag="xres")
                        dma(xres[:], xb_d[jx * 128:(jx + 1) * 128,
                                          ncs:ncs + NCH].bitcast(f32r))
                        rp = rpsum.tile([128, NCH], f32)
                        nc.tensor.matmul(rp[:], resbd_sb[:, half * 128:(half + 1) * 128],
                                         xres[:], start=True, stop=True)
                        nc.vector.scalar_tensor_tensor(yfull[:, ncs:ncs + NCH],
                                                       rp[:], bias3_sb[:, 2:3],
                                                       xt2c[:], ALU.add, ALU.add)
                    # LayerNorm over n (free axis) + relu, mostly in place
                    ssum = stpool.tile([128, 1], f32, tag="ssum")
                    nc.vector.reduce_sum(ssum[:], yfull[:], axis=AX.X)
                    sq = tbpool.tile([128, N], f32, tag="sq", bufs=2,
                                     name=f"sq_{b}_{jt}")
                    sqs = stpool.tile([128, 1], f32, tag="sqs")
                    nc.scalar.activation(sq[:], yfull[:], AF.Square, accum_out=sqs[:])
                    mu = stpool.tile([128, 1], f32, tag="mu")
                    nc.vector.tensor_scalar_mul(mu[:], ssum[:], 1.0 / N)
                    var = stpool.tile([128, 1], f32, tag="var")
                    musq = stpool.tile([128, 1], f32, tag="musq")
                    nc.vector.tensor_mul(musq[:], mu[:], mu[:])
                    nc.vector.tensor_scalar(var[:], sqs[:], 1.0 / N, None, ALU.mult)
                    nc.vector.tensor_sub(var[:], var[:], musq[:])
                    nc.vector.tensor_scalar_add(var[:], var[:], 1e-5)
                    sd = stpool.tile([128, 1], f32, tag="sd")
                    nc.scalar.sqrt(sd[:], var[:])
                    istd = stpool.tile([128, 1], f32, tag="istd")
                    nc.vector.reciprocal(istd[:], sd[:])
                    ynorm = tbpool.tile([128, N], f32, tag="ynorm", bufs=3,
                                        name=f"yn_{b}_{jt}")
                    nc.vector.tensor_scalar(ynorm[:], yfull[:], mu[:], istd[:],
                                            ALU.subtract, ALU.mult)
                    nc.vector.tensor_mul(ynorm[:], ynorm[:], lng_sb[:])
                    nc.vector.tensor_add(ynorm[:], ynorm[:], lnb_sb[:])
                    nc.scalar.activation(ynorm[:], ynorm[:], AF.Relu)
                    dma(y_d[j2 * 128:(j2 + 1) * 128, :], ynorm[:])

    nc.compile()
    return nc


def _host_prep(inputs):
    x = np.asarray(inputs["x"], np.float32)
    cheb = np.asarray(inputs["cheb"], np.float32)
    theta1 = np.asarray(inputs["theta1"], np.float32)
    theta2 = np.asarray(inputs["theta2"], np.float32)
    mlp1_w = np.asarray(inputs["mlp1_w"], np.float32)
    mlp1_b = np.asarray(inputs["mlp1_b"], np.float32)
    mlp2_w = np.asarray(inputs["mlp2_w"], np.float32)
    mlp2_b = np.asarray(inputs["mlp2_b"], np.float32)
    tc1_w = np.asarray(inputs["tc1_w"], np.float32)
    tc1_b = np.asarray(inputs["tc1_b"], np.float32)
    tc2_w = np.asarray(inputs["tc2_w"], np.float32)
    tc2_b = np.asarray(inputs["tc2_b"], np.float32)
    res_w = np.asarray(inputs["res_w"], np.float32)
    res_b = np.asarray(inputs["res_b"], np.float32)
    ln_g = np.asarray(inputs["ln_g"], np.float32)
    ln_b = np.asarray(inputs["ln_b"], np.float32)

    assert np.array_equal(cheb[0], np.eye(N, dtype=np.float32)), \
        "kernel assumes cheb[0] == I"

    th1 = np.zeros((3, 128, 256), np.float32)
    for g in range(8):
        for k in range(3):
            th1[k, g * 16:(g + 1) * 16, g * 32:(g + 1) * 32] = theta1[k]
    th2 = np.zeros((3, 128, 128), np.float32)
    for g in range(4):
        for k in range(3):
            th2[k, g * 32:(g + 1) * 32, g * 32:(g + 1) * 32] = theta2[k]

    src0 = [10] + list(range(11))
    src1 = [11] + list(range(1, 12))
    tcbd = np.zeros((2, 3, 3, 128, 128), np.float32)
    for ti, w in ((0, tc1_w), (1, tc2_w)):
        for tpp in range(12):
            jt, to = divmod(tpp, 4)
            for srcs, kw in ((src0, 0), (src1, 1)):
                tin = srcs[tpp]
                ji, til = divmod(tin, 4)
                tcbd[ti, jt, ji, til * 32:(til + 1) * 32,
                     to * 32:(to + 1) * 32] += w[:, :, 0, kw].T

    resbd = np.zeros((2, 128, 128), np.float32)
    for half in range(2):
        for g4 in range(4):
            g = g4 + 4 * half
            resbd[half, g * 16:(g + 1) * 16,
                  g4 * 32:(g4 + 1) * 32] = res_w[:, :, 0, 0].T

    ident = np.eye(128, dtype=np.float32)
    selA = np.zeros((6, 128, 48), np.float32)
    for j in range(6):
        for p in range(128):
            selA[j, p, 8 * j + p // 16] = 1.0
    selE1 = np.zeros((6, 2, 12, 128), np.float32)
    for j in range(6):
        b0, b1 = B01[j]
        for p in range(128):
            bt = 8 * j + p // 16
            bb, t = divmod(bt, 12)
            selE1[j, 0 if bb == b0 else 1, t, p] = 1.0

    w1aug = np.concatenate([mlp1_w.T, mlp1_b[None]], 0).astype(np.float32)
    w2aug = np.concatenate([mlp2_w.T, mlp2_b[None]], 0).astype(np.float32)
    lng = np.ascontiguousarray(np.broadcast_to(ln_g, (128, N))).astype(np.float32)
    lnb = np.ascontiguousarray(np.broadcast_to(ln_b, (128, N))).astype(np.float32)
    p32 = np.arange(128) % 32
    bias3 = np.stack([tc1_b[p32], tc2_b[p32], res_b[p32],
                      np.zeros(128, np.float32)], axis=1).astype(np.float32)

    shared = dict(cheb1=np.ascontiguousarray(cheb[1]), th1=th1, th2=th2,
                  tcbd=tcbd, resbd=resbd, ident=ident, selA=selA, selE1=selE1,
                  w1aug=w1aug, w2aug=w2aug, lng=lng, lnb=lnb, bias3=bias3)

    in_maps = []
    for c in range(NCORES):
        xc = x[c * BC:(c + 1) * BC]                       # [4, 12, 16, N]
        xT = np.ascontiguousarray(xc.transpose(3, 0, 1, 2).reshape(N, R1))
        xb = np.ascontiguousarray(xc.reshape(R1, N))
        in_maps.append(dict(shared, xT=xT, xb=xb))
    return in_maps


def kernel(**inputs):
    global _compiled
    if _compiled is None:
        _compiled = _build()
    in_maps = _host_prep(inputs)
    res = run_bass_kernel_spmd(_compiled, in_maps, list(range(NCORES)))
    y = np.empty((B, T, O, N), np.float32)
    for c in range(NCORES):
        y[c * BC:(c + 1) * BC] = res.results[c]["y"].reshape(BC, T, O, N)
    return y



# revision 13
# speedup vs baseline: 1.0910x; 1.0910x over previous
"""Trainium2 Bass kernel for nn_Branch1_block (gnn_message_passing).

Data-parallel over batch on 8 NeuronCores (4 batches/core).

Pipeline per core (single compiled graph):
  host:    SE attention scalars, cheb2, blockdiag weights precomputed in numpy
  gconv1:  z_k = x @ cheb_k in bf16 (k=1,2; k=0 from DRAM row-major x),
           att folded as per-partition scale, feature mix in f32r producing
           transposed xg1T [n x (bt,o)] in bf16.
  gconv2:  batch-outer / n-chunk-inner so each batch's output is complete
           early; graph matmuls bf16 (lhsT = xg1T), k=0 via PE transposes,
           feature mix f32r; xg2 stays in SBUF (no DRAM spill).
  tail:    per batch, interleaved under the next batch's gconv2 compute:
           temporal convs as block matmuls (tconv1 bf16, tconv2/residual
           f32r), LayerNorm over nodes fused via ACT Identity(scale,bias).
"""
import sys

import numpy as np

try:
    import concourse.bass as bass
except ImportError:  # pragma: no cover - grading env fallback
    for p in ("/root/.axon_site", "/root/.axon_site/_ro/trn_rl_repo",
              "/root/.axon_site/_ro/pypackages", "/opt/trn_rl_repo"):
        if p not in sys.path:
            sys.path.append(p)
    import concourse.bass as bass

from contextlib import ExitStack

import ml_dtypes
import concourse.mybir as mybir
import concourse.tile as tile
from concourse import bacc
from concourse.bass_utils import run_bass_kernel_spmd

B, T, F, O, N, K = 32, 12, 16, 32, 2048, 3
NCORES = 8
BC = B // NCORES          # 4 batches per core
BT = BC * T               # 48
R1 = BT * F               # 768 rows (bt,f)
R2 = BT * O               # 1536 rows (bt,o)
J1 = R1 // 128            # 6
J2 = R2 // 128            # 12
NT = N // 128             # 16
NCH1 = 256                # gconv1 n-chunk
NNC1 = N // NCH1          # 8
NCH2 = 256                # gconv2 n-chunk
NNC2 = N // NCH2          # 8
TCH = 256                 # tail n-chunk
NTC = N // TCH            # 8

f32 = mybir.dt.float32
f32r = mybir.dt.float32r
bf16 = mybir.dt.bfloat16
AF = mybir.ActivationFunctionType
AX = mybir.AxisListType
ALU = mybir.AluOpType

# (jt, ji) pairs with nonzero temporal-conv block matrices
TC_PAIRS = {0: (0, 2), 1: (0, 1), 2: (1, 2)}

_compiled = {}


def _build(zero_lnb):
    nc = bacc.Bacc("TRN2", target_bir_lowering=False, debug=False)

    xT_d = nc.dram_tensor("xT", [N, R1], bf16, kind="ExternalInput").ap()
    xb_d = nc.dram_tensor("xb", [R1, N], f32, kind="ExternalInput").ap()
    c1b_d = nc.dram_tensor("c1b", [N, N], bf16, kind="ExternalInput").ap()
    c2b_d = nc.dram_tensor("c2b", [N, N], bf16, kind="ExternalInput").ap()
    th1_d = nc.dram_tensor("th1", [3, 128, 256], f32, kind="ExternalInput").ap()
    th2_d = nc.dram_tensor("th2", [3, 128, 128], f32, kind="ExternalInput").ap()
    th2b_d = nc.dram_tensor("th2b", [128, 128], bf16, kind="ExternalInput").ap()
    tcb1_d = nc.dram_tensor("tcb1", [3, 3, 128, 128], bf16,
                            kind="ExternalInput").ap()
    tcb2_d = nc.dram_tensor("tcb2", [3, 3, 128, 128], f32,
                            kind="ExternalInput").ap()
    resbd_d = nc.dram_tensor("resbd", [2, 128, 128], f32, kind="ExternalInput").ap()
    identb_d = nc.dram_tensor("identb", [128, 128], bf16, kind="ExternalInput").ap()
    attc_d = nc.dram_tensor("attc", [128, 8], f32, kind="ExternalInput").ap()
    lng_d = nc.dram_tensor("lng", [128, N], f32, kind="ExternalInput").ap()
    lnb_d = nc.dram_tensor("lnb", [128, N], f32, kind="ExternalInput").ap()
    bias3_d = nc.dram_tensor("bias3", [128, 4], f32, kind="ExternalInput").ap()
    y_d = nc.dram_tensor("y", [R2, N], f32, kind="ExternalOutput").ap()

    dma = nc.sync.dma_start

    with tile.TileContext(nc) as tc, ExitStack() as top:
        cpool = top.enter_context(tc.tile_pool(name="const", bufs=1))
        identb_sb = cpool.tile([128, 128], bf16)
        th2_sb = cpool.tile([128, 3 * 128], f32r)
        th2b_sb = cpool.tile([128, 128], bf16)
        attc_sb = cpool.tile([128, 8], f32)
        dma(identb_sb[:], identb_d)
        dma(th2_sb[:].rearrange("p (k c) -> p k c", c=128),
            th2_d.rearrange("k p c -> p k c").bitcast(f32r))
        dma(th2b_sb[:], th2b_d)
        dma(attc_sb[:], attc_d)

        xg1Tp = top.enter_context(tc.tile_pool(name="xg1T", bufs=1))
        xg1T_sb = xg1Tp.tile([128, NT, R2], bf16)

        # ---------- gconv1 ----------
        with tc.tile_pool(name="xTp", bufs=1) as xTpool, \
             tc.tile_pool(name="c1", bufs=1) as c1pool, \
             tc.tile_pool(name="chp", bufs=2) as chpool, \
             tc.tile_pool(name="g1sb", bufs=2) as g1pool, \
             tc.tile_pool(name="zps", bufs=3, space="PSUM") as zpsum, \
             tc.tile_pool(name="fps", bufs=2, space="PSUM") as fpsum:
            th1_sb = c1pool.tile([128, 3 * 256], f32r)
            dma(th1_sb[:].rearrange("p (k c) -> p k c", c=256),
                th1_d.rearrange("k p c -> p k c").bitcast(f32r))
            xT_sb = xTpool.tile([128, NT, R1], bf16)
            xTv = xT_d.rearrange("(mi p) r -> mi p r", p=128)
            for q in range(8):
                dma(xT_sb[:, q * 2:(q + 1) * 2, :],
                    xTv[q * 2:(q + 1) * 2].rearrange("mi p r -> p mi r"))

            for nci in range(NNC1):
                ncs = nci * NCH1
                ch1 = chpool.tile([128, NT, NCH1], bf16, tag="ch1", bufs=2,
                                  name=f"ch1_{nci}")
                ch2 = chpool.tile([128, NT, NCH1], bf16, tag="ch2", bufs=2,
                                  name=f"ch2_{nci}")
                for cd, ct in ((c1b_d, ch1), (c2b_d, ch2)):
                    chv = cd[:, ncs:ncs + NCH1].rearrange("(mi p) n -> mi p n",
                                                          p=128)
                    for q in range(2):
                        dma(ct[:, q * 8:(q + 1) * 8, :],
                            chv[q * 8:(q + 1) * 8].rearrange("mi p n -> p mi n"))
                zk = {}
                for k, ch in ((1, ch1), (2, ch2)):
                    for j1 in range(J1):
                        zps = zpsum.tile([128, NCH1], f32)
                        for mi in range(NT):
                            nc.tensor.matmul(zps[:],
                                             xT_sb[:, mi, j1 * 128:(j1 + 1) * 128],
                                             ch[:, mi, :],
                                             start=(mi == 0), stop=(mi == NT - 1))
                        zt = g1pool.tile([128, NCH1], f32r, tag=f"z{k}",
                                         bufs=(7 if k == 1 else 3))
                        nc.vector.tensor_scalar_mul(zt[:], zps[:],
                                                    attc_sb[:, j1:j1 + 1])
                        zk[(k, j1)] = zt
                        if k == 2:
                            z0t = g1pool.tile([128, NCH1], f32r, tag="z0", bufs=3)
                            dma(z0t[:], xb_d[j1 * 128:(j1 + 1) * 128,
                                             ncs:ncs + NCH1].bitcast(f32r))
                            nc.vector.tensor_scalar_mul(z0t[:], z0t[:].bitcast(f32),
                                                        attc_sb[:, j1:j1 + 1])
                            for ntl in range(NCH1 // 128):
                                ntile = (ncs // 128) + ntl
                                fps = fpsum.tile([128, 256], f32)
                                nc.tensor.matmul(fps[:], z0t[:, ntl * 128:(ntl + 1) * 128],
                                                 th1_sb[:, 0:256], start=True, stop=False)
                                nc.tensor.matmul(fps[:],
                                                 zk[(1, j1)][:, ntl * 128:(ntl + 1) * 128],
                                                 th1_sb[:, 256:512],
                                                 start=False, stop=False)
                                nc.tensor.matmul(fps[:],
                                                 zk[(2, j1)][:, ntl * 128:(ntl + 1) * 128],
                                                 th1_sb[:, 512:768],
                                                 start=False, stop=True)
                                nc.scalar.activation(
                                    xg1T_sb[:, ntile, j1 * 256:(j1 + 1) * 256],
                                    fps[:], AF.Relu)

        # ---------- gconv2 + tail, interleaved per batch ----------
        with tc.tile_pool(name="c3", bufs=1) as c3pool, \
             tc.tile_pool(name="chp2", bufs=2) as chpool2, \
             tc.tile_pool(name="g2sb", bufs=2) as g2pool, \
             tc.tile_pool(name="xg2p", bufs=2) as xg2pool, \
             tc.tile_pool(name="tlsb", bufs=3) as tlpool, \
             tc.tile_pool(name="tbig", bufs=2) as tbpool, \
             tc.tile_pool(name="tstat", bufs=2) as stpool:
            tcb1_sb = c3pool.tile([128, 9 * 128], bf16)
            tcb2_sb = c3pool.tile([128, 9 * 128], f32r)
            resbd_sb = c3pool.tile([128, 2 * 128], f32r)
            lng_sb = c3pool.tile([128, N], f32)
            bias3_sb = c3pool.tile([128, 4], f32)
            dma(tcb1_sb[:].rearrange("p (i q) -> p i q", q=128),
                tcb1_d.rearrange("a b p q -> p (a b) q"))
            dma(tcb2_sb[:].rearrange("p (i q) -> p i q", q=128),
                tcb2_d.rearrange("a b p q -> p (a b) q").bitcast(f32r))
            dma(resbd_sb[:].rearrange("p (h q) -> p h q", q=128),
                resbd_d.rearrange("h p q -> p h q").bitcast(f32r))
            dma(lng_sb[:], lng_d)
            dma(bias3_sb[:], bias3_d)
            if not zero_lnb:
                lnb_sb = c3pool.tile([128, N], f32)
                dma(lnb_sb[:], lnb_d)

            for b in range(BC):
                xg2b = [xg2pool.tile([128, N], bf16, tag="xg2", bufs=5,
                                     name=f"xg2_{b}_{i}") for i in range(3)]
                g2ps = ExitStack()
                zpsum2 = g2ps.enter_context(
                    tc.tile_pool(name=f"zps2_{b}", bufs=2, space="PSUM"))
                tpsumT = g2ps.enter_context(
                    tc.tile_pool(name=f"tpsT_{b}", bufs=2, space="PSUM"))
                fpsum2 = g2ps.enter_context(
                    tc.tile_pool(name=f"fps2_{b}", bufs=2, space="PSUM"))
                for nci in range(NNC2):
                    ncs = nci * NCH2
                    cb1 = chpool2.tile([128, NT, NCH2], bf16, tag="cb1", bufs=2,
                                       name=f"cb1_{b}_{nci}")
                    cb2 = chpool2.tile([128, NT, NCH2], bf16, tag="cb2", bufs=2,
                                       name=f"cb2_{b}_{nci}")
                    for cd, ct in ((c1b_d, cb1), (c2b_d, cb2)):
                        chv = cd[:, ncs:ncs + NCH2].rearrange(
                            "(mi p) n -> mi p n", p=128)
                        for q in range(2):
                            dma(ct[:, q * 8:(q + 1) * 8, :],
                                chv[q * 8:(q + 1) * 8].rearrange("mi p n -> p mi n"))
                    zrec = {}
                    xg1r = {}
                    for jl in range(3):
                        j2 = 3 * b + jl
                        for k, cb in ((1, cb1), (2, cb2)):
                            zps = zpsum2.tile([128, NCH2], f32)
                            for mi in range(NT):
                                nc.tensor.matmul(
                                    zps[:], xg1T_sb[:, mi, j2 * 128:(j2 + 1) * 128],
                                    cb[:, mi, :],
                                    start=(mi == 0), stop=(mi == NT - 1))
                            zt = g2pool.tile([128, NCH2], f32r, tag=f"z2_{k}",
                                             bufs=4)
                            nc.vector.tensor_copy(zt[:], zps[:])
                            zrec[(jl, k)] = zt
                        xr = g2pool.tile([128, NCH2], bf16, tag="xg1r", bufs=4)
                        for tl in range(NCH2 // 128):
                            mi = (ncs // 128) + tl
                            tp = tpsumT.tile([128, 128], bf16)
                            nc.tensor.transpose(
                                tp[:], xg1T_sb[:, mi, j2 * 128:(j2 + 1) * 128],
                                identb_sb[:])
                            nc.vector.tensor_copy(xr[:, tl * 128:(tl + 1) * 128],
                                                  tp[:])
                        xg1r[jl] = xr
                    for jl in range(3):
                        fps = fpsum2.tile([128, NCH2], f32)
                        nc.tensor.matmul(fps[:], th2b_sb[:], xg1r[jl][:],
                                         start=True, stop=False)
                        nc.tensor.matmul(fps[:], th2_sb[:, 128:256],
                                         zrec[(jl, 1)][:], start=False, stop=False)
                        nc.tensor.matmul(fps[:], th2_sb[:, 256:384],
                                         zrec[(jl, 2)][:], start=False, stop=True)
                        nc.scalar.activation(xg2b[jl][:, ncs:ncs + NCH2],
                                             fps[:], AF.Relu)

                # ---- tail for batch b ----
                g2ps.close()
                tlps = ExitStack()
                tpsum1 = tlps.enter_context(
                    tc.tile_pool(name=f"tps1_{b}", bufs=2, space="PSUM"))
                tpsum2 = tlps.enter_context(
                    tc.tile_pool(name=f"tps2_{b}", bufs=2, space="PSUM"))
                rpsum = tlps.enter_context(
                    tc.tile_pool(name=f"rps_{b}", bufs=2, space="PSUM"))
                xt1b = [tbpool.tile([128, N], f32r, tag="xt1", bufs=3,
                                    name=f"xt1_{b}_{i}") for i in range(3)]
                for jt in range(3):
                    ja, jb = TC_PAIRS[jt]
                    for ncc in range(NTC):
                        ncs = ncc * TCH
                        tp1 = tpsum1.tile([128, TCH], f32)
                        nc.tensor.matmul(tp1[:],
                                         tcb1_sb[:, (jt * 3 + ja) * 128:
                                                 (jt * 3 + ja + 1) * 128],
                                         xg2b[ja][:, ncs:ncs + TCH],
                                         start=True, stop=False)
                        nc.tensor.matmul(tp1[:],
                                         tcb1_sb[:, (jt * 3 + jb) * 128:
                                                 (jt * 3 + jb + 1) * 128],
                                         xg2b[jb][:, ncs:ncs + TCH],
                                         start=False, stop=True)
                        nc.scalar.activation(xt1b[jt][:, ncs:ncs + TCH], tp1[:],
                                             AF.Relu, bias=bias3_sb[:, 0:1])
                for jt in range(3):
                    j2 = 3 * b + jt
                    ja, jb = TC_PAIRS[jt]
                    jx, half = divmod(j2, 2)
                    xres = tbpool.tile([128, N], f32r, tag="xres", bufs=2,
                                       name=f"xres_{b}_{jt}")
                    dma(xres[:], xb_d[jx * 128:(jx + 1) * 128, :].bitcast(f32r))
                    yfull = tbpool.tile([128, N], f32, tag="yfull", bufs=2,
                                        name=f"yf_{b}_{jt}")
                    for ncc in range(NTC):
                        ncs = ncc * TCH
                        tp2 = tpsum2.tile([128, TCH], f32)
                        nc.tensor.matmul(tp2[:],
                                         tcb2_sb[:, (jt * 3 + ja) * 128:
                                                 (jt * 3 + ja + 1) * 128],
                                         xt1b[ja][:, ncs:ncs + TCH],
                                         start=True, stop=False)
                        nc.tensor.matmul(tp2[:],
                                         tcb2_sb[:, (jt * 3 + jb) * 128:
                                                 (jt * 3 + jb + 1) * 128],
                                         xt1b[jb][:, ncs:ncs + TCH],
                                         start=False, stop=True)
                        xt2c = tlpool.tile([128, TCH], f32, tag="xt2c", bufs=3)
                        nc.scalar.activation(xt2c[:], tp2[:], AF.Relu,
                                             bias=bias3_sb[:, 1:2])
                        rp = rpsum.tile([128, TCH], f32)
                        nc.tensor.matmul(rp[:],
                                         resbd_sb[:, half * 128:(half + 1) * 128],
                                         xres[:, ncs:ncs + TCH],
                                         start=True, stop=True)
                        nc.vector.scalar_tensor_tensor(yfull[:, ncs:ncs + TCH],
                                                       rp[:], bias3_sb[:, 2:3],
                                                       xt2c[:], ALU.add, ALU.add)
                    # LayerNorm over n (free axis) + relu
                    ssum = stpool.tile([128, 1], f32, tag="ssum")
                    nc.vector.reduce_sum(ssum[:], yfull[:], axis=AX.X)
                    scr = tbpool.tile([128, N], f32, tag="scr", bufs=2,
                                      name=f"scr_{b}_{jt}")
                    sqs = stpool.tile([128, 1], f32, tag="sqs")
                    nc.scalar.activation(scr[:], yfull[:], AF.Square,
                                         accum_out=sqs[:])
                    mu = stpool.tile([128, 1], f32, tag="mu")
                    nc.vector.tensor_scalar_mul(mu[:], ssum[:], 1.0 / N)
                    musq = stpool.tile([128, 1], f32, tag="musq")
                    nc.vector.tensor_mul(musq[:], mu[:], mu[:])
                    var = stpool.tile([128, 1], f32, tag="var")
                    nc.vector.tensor_scalar(var[:], sqs[:], 1.0 / N, None, ALU.mult)
                    nc.vector.tensor_sub(var[:], var[:], musq[:])
                    nc.vector.tensor_scalar_add(var[:], var[:], 1e-5)
                    sd = stpool.tile([128, 1], f32, tag="sd")
                    nc.scalar.sqrt(sd[:], var[:])
                    istd = stpool.tile([128, 1], f32, tag="istd")
                    nc.vector.reciprocal(istd[:], sd[:])
                    nmi = stpool.tile([128, 1], f32, tag="nmi")
                    nc.vector.scalar_tensor_tensor(nmi[:], mu[:], -1.0, istd[:],
                                                   ALU.mult, ALU.mult)
                    # t = yfull*istd - mu*istd  (one ACT op), then *g (+b), relu
                    nc.scalar.activation(scr[:], yfull[:], AF.Identity,
                                         bias=nmi[:, 0:1], scale=istd[:, 0:1])
                    nc.vector.tensor_mul(scr[:], scr[:], lng_sb[:])
                    if not zero_lnb:
                        nc.vector.tensor_add(scr[:], scr[:], lnb_sb[:])
                    nc.scalar.activation(scr[:], scr[:], AF.Relu)
                    dma(y_d[j2 * 128:(j2 + 1) * 128, :], scr[:])
                tlps.close()

    nc.compile()
    return nc


def _host_prep(inputs):
    x = np.asarray(inputs["x"], np.float32)
    cheb = np.asarray(inputs["cheb"], np.float32)
    theta1 = np.asarray(inputs["theta1"], np.float32)
    theta2 = np.asarray(inputs["theta2"], np.float32)
    mlp1_w = np.asarray(inputs["mlp1_w"], np.float32)
    mlp1_b = np.asarray(inputs["mlp1_b"], np.float32)
    mlp2_w = np.asarray(inputs["mlp2_w"], np.float32)
    mlp2_b = np.asarray(inputs["mlp2_b"], np.float32)
    tc1_w = np.asarray(inputs["tc1_w"], np.float32)
    tc1_b = np.asarray(inputs["tc1_b"], np.float32)
    tc2_w = np.asarray(inputs["tc2_w"], np.float32)
    tc2_b = np.asarray(inputs["tc2_b"], np.float32)
    res_w = np.asarray(inputs["res_w"], np.float32)
    res_b = np.asarray(inputs["res_b"], np.float32)
    ln_g = np.asarray(inputs["ln_g"], np.float32)
    ln_b = np.asarray(inputs["ln_b"], np.float32)

    assert np.array_equal(cheb[0], np.eye(N, dtype=np.float32)), \
        "kernel assumes cheb[0] == I"

    # SE attention on host: att[b,t] = sigmoid(relu(mean @ W1.T + b1) @ W2.T + b2)
    am = x.mean(axis=(2, 3))
    a1 = np.maximum(am @ mlp1_w.T + mlp1_b, 0.0)
    att = 1.0 / (1.0 + np.exp(-(a1 @ mlp2_w.T + mlp2_b)))
    att = att.astype(np.float32)

    c1b = cheb[1].astype(ml_dtypes.bfloat16)
    c2b = cheb[2].astype(ml_dtypes.bfloat16)

    th1 = np.zeros((3, 128, 256), np.float32)
    for g in range(8):
        for k in range(3):
            th1[k, g * 16:(g + 1) * 16, g * 32:(g + 1) * 32] = theta1[k]
    th2 = np.zeros((3, 128, 128), np.float32)
    for g in range(4):
        for k in range(3):
            th2[k, g * 32:(g + 1) * 32, g * 32:(g + 1) * 32] = theta2[k]
    th2b = th2[0].astype(ml_dtypes.bfloat16)

    src0 = [10] + list(range(11))
    src1 = [11] + list(range(1, 12))
    tcbd = np.zeros((2, 3, 3, 128, 128), np.float32)
    for ti, w in ((0, tc1_w), (1, tc2_w)):
        for tpp in range(12):
            jt, to = divmod(tpp, 4)
            for srcs, kw in ((src0, 0), (src1, 1)):
                tin = srcs[tpp]
                ji, til = divmod(tin, 4)
                tcbd[ti, jt, ji, til * 32:(til + 1) * 32,
                     to * 32:(to + 1) * 32] += w[:, :, 0, kw].T
    tcb1 = tcbd[0].astype(ml_dtypes.bfloat16)
    tcb2 = tcbd[1]

    resbd = np.zeros((2, 128, 128), np.float32)
    for half in range(2):
        for g4 in range(4):
            g = g4 + 4 * half
            resbd[half, g * 16:(g + 1) * 16,
                  g4 * 32:(g4 + 1) * 32] = res_w[:, :, 0, 0].T

    identb = np.eye(128, dtype=ml_dtypes.bfloat16)
    lng = np.ascontiguousarray(np.broadcast_to(ln_g, (128, N))).astype(np.float32)
    lnb = np.ascontiguousarray(np.broadcast_to(ln_b, (128, N))).astype(np.float32)
    p32 = np.arange(128) % 32
    bias3 = np.stack([tc1_b[p32], tc2_b[p32], res_b[p32],
                      np.zeros(128, np.float32)], axis=1).astype(np.float32)

    shared = dict(c1b=c1b, c2b=c2b, th1=th1, th2=th2, th2b=th2b, tcb1=tcb1,
                  tcb2=tcb2, resbd=resbd, identb=identb, lng=lng, lnb=lnb,
                  bias3=bias3)

    in_maps = []
    for c in range(NCORES):
        xc = x[c * BC:(c + 1) * BC]                       # [4, 12, 16, N]
        xT = np.ascontiguousarray(
            xc.transpose(3, 0, 1, 2).reshape(N, R1)).astype(ml_dtypes.bfloat16)
        xb = np.ascontiguousarray(xc.reshape(R1, N))
        attc = np.zeros((128, 8), np.float32)
        for j in range(J1):
            for p in range(128):
                bt = 8 * j + p // 16
                attc[p, j] = att[c * BC + bt // T, bt % T]
        in_maps.append(dict(shared, xT=xT, xb=xb, attc=attc))
    return in_maps


def _zero_lnb(inputs):
    return bool(np.all(np.asarray(inputs["ln_b"]) == 0.0))


def kernel(**inputs):
    zl = _zero_lnb(inputs)
    if zl not in _compiled:
        _compiled[zl] = _build(zl)
    in_maps = _host_prep(inputs)
    res = run_bass_kernel_spmd(_compiled[zl], in_maps, list(range(NCORES)))
    y = np.empty((B, T, O, N), np.float32)
    for c in range(NCORES):
        y[c * BC:(c + 1) * BC] = res.results[c]["y"].reshape(BC, T, O, N)
    return y
